# revision 1
# baseline (speedup 1.0000x reference)
"""CaNetConv (GCN conv + gated multi-head linear) Trainium2 kernel.

Strategy (pull-mode graph SpMM, destinations sharded across 8 cores):
  hi[c,:] = sum_{e: col[e]=c} val[e] * x[row[e],:]      (GCN aggregation)
  out     = x + sum_k e[:,k] * (concat(hi,x) @ W[k])    (gated einsum)

Per core:
  - edges sorted by destination; 128-edge groups; x rows fetched with
    gpsimd dma_gather (512B tokens -> full DMA line rate)
  - DVE builds S[e,c] = (iota==colrel[e]) * val[e] in one tensor_scalar
  - PE computes hiT[f,c] += msg[e,f]^T @ S[e,c] (segment sum as matmul,
    accumulated in PSUM per 128-dest block)
  - einsum as 4 matmuls per 128-node block vs preflattened W, gating sum
    via scalar_tensor_tensor with per-partition e scalars, + residual.

One NEFF shared by all 8 cores (SPMD): the static structure (gather run
lengths, groups per block) is padded to the max across cores.
"""

import sys

import numpy as np

for _p in ("/opt/trn_rl_repo", "/root/.axon_site/_ro/trn_rl_repo"):
    if _p not in sys.path:
        sys.path.append(_p)

import concourse.bass as bass  # noqa: E402
import concourse.tile as tile  # noqa: E402
from concourse import bacc, mybir  # noqa: E402

F32 = mybir.dt.float32
I16 = mybir.dt.int16

SUBRUN = 1024          # gather tokens per dma_gather; the SWDGE descriptor
                       # carveout (16KB/partition / 16B per desc) caps one
                       # gather at ~1024 descriptors — 1280 kills the device
SB_BLOCKS = 4          # dest blocks (128 dests) per gather superblock
PAD_COLREL = 200.0     # colrel sentinel that never matches iota 0..127


def _wrap16(a):
    """dma_gather index layout: [128, n/16], idx t at [t%16 (+16g), t//16]."""
    n = a.shape[0]
    assert n % 16 == 0
    w = a.reshape(n // 16, 16).T.astype(np.int16)  # [16, n/16]
    return np.tile(w, (8, 1))                      # replicated per Q7 core


def _prep(x, adj, e, weights, n_cores):
    """Host-side graph preprocessing. Returns (meta, in_maps)."""
    N, F = x.shape
    K = e.shape[1]
    E = adj.shape[1]
    row = np.asarray(adj[0], dtype=np.int64)
    col = np.asarray(adj[1], dtype=np.int64)

    NPC = N // n_cores                      # dests per core
    NB = (NPC + 127) // 128                 # 128-dest blocks per core
    NPCP = NB * 128
    NSB = (NB + SB_BLOCKS - 1) // SB_BLOCKS
    sb_nblocks = [min(SB_BLOCKS, NB - s * SB_BLOCKS) for s in range(NSB)]
    import os as _os
    n_half = 2 if (N > 32767 or _os.environ.get("KERNEL_FORCE_HALVES")) else 1
    HALF = (N + 1) // 2 if n_half == 2 else N

    # GCN normalization (destination degree), f32 like the reference
    deg = np.bincount(col, minlength=N).astype(np.float32)
    with np.errstate(divide="ignore"):
        r = 1.0 / np.sqrt(deg)
    r[~np.isfinite(r)] = 0.0
    val_e = (r[col] * r[row]).astype(np.float32)

    # per-core edge lists sorted by local dest
    cores = []
    counts = np.zeros((n_cores, NB, n_half), dtype=np.int64)
    for c in range(n_cores):
        m = (col >= c * NPC) & (col < (c + 1) * NPC)
        rc = row[m]
        cc = col[m] - c * NPC
        vc = val_e[m]
        o = np.argsort(cc, kind="stable")
        rc, cc, vc = rc[o], cc[o], vc[o]
        blk = cc >> 7
        half = (rc >= HALF).astype(np.int64)
        # bucket edges per (block, half), preserving order
        key = blk * n_half + half
        ob = np.argsort(key, kind="stable")
        rc, cc, vc, blk, half = rc[ob], cc[ob], vc[ob], blk[ob], half[ob]
        np.add.at(counts[c], (blk, half), 1)
        cores.append((rc, cc, vc, blk, half))

    # static structure: groups per (block, half) = max over cores
    G = np.maximum.reduce([np.ceil(counts[c] / 128.0).astype(np.int64)
                           for c in range(n_cores)])
    for b in range(NB):
        if G[b].sum() == 0:
            G[b, 0] = 1  # guarantee every block's psum region is written

    # Token stream AND matmul emission are block-major (for b: for h:
    # groups): each (block, half) span is one gather run (single source
    # half), each block's PSUM accumulation group is contiguous, and msg
    # tiles are consumed in stream order (bounded pool pressure).
    sched = []          # per sb: [(token_gidx, b_loc, start, stop), ...]
    sb_runs = []        # per sb: [(h, ntokens), ...] gather runs in order
    for s in range(NSB):
        blocks = list(range(s * SB_BLOCKS, s * SB_BLOCKS + sb_nblocks[s]))
        groups = []
        runs = []
        for bi, b in enumerate(blocks):
            # alternate the half order per block so adjacent blocks' runs
            # share a source half and merge into one gather run below
            order = ((0, 1) if (bi % 2 == 0 or n_half == 1) else (1, 0))[:n_half]
            metas = []
            for h in order:
                if G[b, h] > 0:
                    if runs and runs[-1][0] == h:
                        runs[-1][1] += int(G[b, h]) * 128
                    else:
                        runs.append([h, int(G[b, h]) * 128])
                for _ in range(G[b, h]):
                    metas.append([len(groups) + len(metas),
                                  b - s * SB_BLOCKS, False, False])
            metas[0][2] = True
            metas[-1][3] = True
            groups.extend(metas)
        sched.append(groups)
        sb_runs.append(runs)

    G_total = sum(len(g) for g in sched)

    # per-core token arrays in schedule order
    in_maps = []
    xpad = np.zeros((N + 128, F), dtype=np.float32)
    xpad[:N] = x
    epad = np.zeros((N + 128, K), dtype=np.float32)
    epad[:N] = e
    WF = np.ascontiguousarray(
        weights.astype(np.float32).transpose(1, 0, 2).reshape(2 * F, K * F))
    W_dram = np.concatenate([WF[:F], WF[F:]], axis=1)  # [128, 2*K*F]

    for c in range(n_cores):
        rc, cc, vc, blk, half = cores[c]
        idx_parts, colrel_parts, val_parts = [], [], []
        for s in range(NSB):
            blocks = range(s * SB_BLOCKS, s * SB_BLOCKS + sb_nblocks[s])
            for bi, b in enumerate(blocks):
                for h in ((0, 1) if (bi % 2 == 0 or n_half == 1) else (1, 0))[:n_half]:
                    m = (blk == b) & (half == h)
                    ridx = rc[m] - h * HALF
                    crel = (cc[m] - b * 128).astype(np.float32)
                    v = vc[m]
                    n = m.sum()
                    npad = G[b, h] * 128 - n
                    assert npad >= 0
                    idx_parts.append(np.concatenate(
                        [ridx, np.zeros(npad, np.int64)]).astype(np.int16))
                    colrel_parts.append(np.concatenate(
                        [crel, np.full(npad, PAD_COLREL, np.float32)]))
                    val_parts.append(np.concatenate(
                        [v, np.zeros(npad, np.float32)]))
        idx_cat = np.concatenate(idx_parts)
        colrel_cat = np.concatenate(colrel_parts)
        val_cat = np.concatenate(val_parts)
        assert idx_cat.shape[0] == G_total * 128

        # wrapped idx stream, sliced per (sb, block, half) gather sub-run
        idx_w = []
        off = 0
        for s in range(NSB):
            for _h, ntok in sb_runs[s]:
                rem = ntok
                while rem > 0:
                    take = min(SUBRUN, rem)
                    idx_w.append(_wrap16(idx_cat[off:off + take]))
                    off += take
                    rem -= take
        idx_dram = (np.concatenate(idx_w, axis=1) if idx_w
                    else np.zeros((128, 16), np.int16))

        colrel_dram = np.ascontiguousarray(
            colrel_cat.reshape(G_total, 128).T)
        val_dram = np.ascontiguousarray(val_cat.reshape(G_total, 128).T)

        xT = np.ascontiguousarray(xpad[c * NPC:c * NPC + NPCP].T)
        x_res = np.ascontiguousarray(xpad[c * NPC:c * NPC + NPCP])
        e_gate = np.ascontiguousarray(
            epad[c * NPC:c * NPC + NPCP].reshape(NB, 128, K)
            .transpose(1, 0, 2).reshape(128, NB * K))

        in_maps.append({
            "x_full": np.ascontiguousarray(xpad),
            "xT": xT,
            "x_res": x_res,
            "e_gate": e_gate,
            "W": np.ascontiguousarray(W_dram),
            "iota": np.tile(np.arange(128, dtype=np.float32), (128, 1)),
            "idx": np.ascontiguousarray(idx_dram),
            "colrel": colrel_dram,
            "val": val_dram,
            "chain": np.zeros((1, 128), np.float32),
        })

    meta = dict(N=N, F=F, K=K, E=E, NPC=NPC, NB=NB, NPCP=NPCP, NSB=NSB,
                sb_nblocks=sb_nblocks, n_half=n_half, HALF=HALF,
                sched=sched, sb_runs=sb_runs, G_total=G_total,
                idx_cols=in_maps[0]["idx"].shape[1],
                n_cores=n_cores)
    return meta, in_maps


def _build(meta):
    """Trace the Bass/Tile kernel for the static structure in meta."""
    from contextlib import ExitStack

    N, F, K = meta["N"], meta["F"], meta["K"]
    NB, NPCP, NSB = meta["NB"], meta["NPCP"], meta["NSB"]
    sb_nblocks, n_half, HALF = meta["sb_nblocks"], meta["n_half"], meta["HALF"]
    sched, sb_runs = meta["sched"], meta["sb_runs"]

    nc = bacc.Bacc("TRN2", target_bir_lowering=False, debug=False,
                   num_devices=meta["n_cores"], num_swdge_queues=4)

    x_full = nc.dram_tensor("x_full", [N + 128, F], F32, kind="ExternalInput")
    xT_d = nc.dram_tensor("xT", [128, NPCP], F32, kind="ExternalInput")
    x_res_d = nc.dram_tensor("x_res", [NPCP, F], F32, kind="ExternalInput")
    e_gate_d = nc.dram_tensor("e_gate", [128, NB * K], F32,
                              kind="ExternalInput")
    KF = K * F
    W_d = nc.dram_tensor("W", [128, 2 * KF], F32, kind="ExternalInput")
    iota_d = nc.dram_tensor("iota", [128, 128], F32, kind="ExternalInput")
    idx_d = nc.dram_tensor("idx", [128, meta["idx_cols"]], I16,
                           kind="ExternalInput")
    colrel_d = nc.dram_tensor("colrel", [128, meta["G_total"]], F32,
                              kind="ExternalInput")
    val_d = nc.dram_tensor("val", [128, meta["G_total"]], F32,
                           kind="ExternalInput")
    out_d = nc.dram_tensor("out", [NPCP, F], F32, kind="ExternalOutput")
    # tiny chain tensors so a benchmark can sequence K executions of this
    # NEFF inside one jit call (data dependency defeats CSE/DCE)
    chain_i = nc.dram_tensor("chain", [1, 128], F32, kind="ExternalInput")
    chain_o = nc.dram_tensor("chain_out", [1, 128], F32,
                             kind="ExternalOutput")

    W_COLS = 2 * KF
    assert KF == 1024 and F == 128, "einsum slicing hardcoded for K=8, F=128"

    with tile.TileContext(nc) as tc, ExitStack() as ctx:
        const = ctx.enter_context(tc.tile_pool(name="const", bufs=1))
        msgp = ctx.enter_context(tc.tile_pool(name="msg", bufs=10))
        sp = ctx.enter_context(tc.tile_pool(name="sp", bufs=16))
        hiTp = ctx.enter_context(tc.tile_pool(name="hiT", bufs=NSB))
        accp = ctx.enter_context(tc.tile_pool(name="acc", bufs=6))
        psag = ctx.enter_context(tc.tile_pool(name="psag", bufs=3,
                                              space="PSUM"))
        psmm = ctx.enter_context(tc.tile_pool(name="psmm", bufs=4,
                                              space="PSUM"))

        # idx cols consumed by each superblock's gathers (for chunked loads)
        sb_idx_cols = []
        for s in range(NSB):
            cols = 0
            for _h, ntok in sb_runs[s]:
                rem = ntok
                while rem > 0:
                    take = min(SUBRUN, rem)
                    cols += take // 16
                    rem -= take
            sb_idx_cols.append(cols)

        # persistent tiles; idx is loaded in per-superblock chunks (first
        # chunk first) so early gathers don't wait on the full idx stream
        # or the other constant loads.
        idx_t = const.tile([128, meta["idx_cols"]], I16, tag="idx")
        iota_t = const.tile([128, 128], F32, tag="iota")
        cr_t = const.tile([128, meta["G_total"]], F32, tag="cr")
        val_t = const.tile([128, meta["G_total"]], F32, tag="val")
        w_t = const.tile([128, W_COLS], F32, tag="w")
        xT_t = const.tile([128, NPCP], F32, tag="xT")
        eg_t = const.tile([128, NB * K], F32, tag="eg")
        off = 0
        for s in range(NSB):
            if sb_idx_cols[s]:
                nc.sync.dma_start(idx_t[:, off:off + sb_idx_cols[s]],
                                  idx_d.ap()[:, off:off + sb_idx_cols[s]])
                off += sb_idx_cols[s]
            if s == 0:
                nc.sync.dma_start(iota_t[:], iota_d.ap()[:, :])
                nc.sync.dma_start(cr_t[:], colrel_d.ap()[:, :])
                nc.sync.dma_start(val_t[:], val_d.ap()[:, :])
                nc.sync.dma_start(w_t[:], W_d.ap()[:, :])
                nc.sync.dma_start(xT_t[:], xT_d.ap()[:, :])
                nc.sync.dma_start(eg_t[:], e_gate_d.ap()[:, :])

        x_half = [x_full.ap()[h * HALF:N + 128, :] for h in range(n_half)]

        chp = ctx.enter_context(tc.tile_pool(name="chp", bufs=1))
        ch_t = chp.tile([1, 128], F32, tag="ch")
        nc.sync.dma_start(ch_t[:], chain_i.ap()[:, :])
        nc.sync.dma_start(chain_o.ap()[:, :], ch_t[:])

        g_base = 0
        idx_off = 0
        n_gathers = 0
        for s in range(NSB):
            nb = sb_nblocks[s]
            groups = sched[s]
            ps_hi = psag.tile([128, nb * 128], F32, tag="psag")

            # gather msg tiles: one run per (block, half), in SUBRUN chunks
            tok_map = []            # token-order group idx -> (tile, slot)
            for h, ntok in sb_runs[s]:
                rem = ntok
                while rem > 0:
                    take = min(SUBRUN, rem)
                    mt = msgp.tile([128, take // 128, 128], F32, tag="msg")
                    nc.gpsimd.dma_gather(
                        mt[:], x_half[h],
                        idx_t[:, idx_off:idx_off + take // 16],
                        take, take, F, queue_num=n_gathers % 4)
                    n_gathers += 1
                    for j in range(take // 128):
                        tok_map.append((mt, j))
                    idx_off += take // 16
                    rem -= take

            # S build + aggregation matmuls, block-major emission order
            for tg, b_loc, start, stop in groups:
                g = g_base + tg
                s_t = sp.tile([128, 128], F32, tag="s")
                nc.vector.tensor_scalar(
                    s_t[:], iota_t[:], cr_t[:, g:g + 1], val_t[:, g:g + 1],
                    mybir.AluOpType.is_equal, mybir.AluOpType.mult)
                mt, j = tok_map[tg]
                nc.tensor.matmul(
                    ps_hi[:, b_loc * 128:(b_loc + 1) * 128],
                    mt[:, j:j + 1, :], s_t[:], start=start, stop=stop)
            g_base += len(groups)

            hiT_t = hiTp.tile([128, nb * 128], F32, tag="hiT")
            nc.vector.tensor_copy(hiT_t[:], ps_hi[:])

            import os as _os
            if _os.environ.get("KERNEL_SKIP_EINSUM"):
                for b_loc in range(nb):
                    b = s * SB_BLOCKS + b_loc
                    acc = accp.tile([128, F], F32, tag="acc")
                    nc.vector.tensor_copy(acc[:],
                                          hiT_t[:, b_loc * 128:(b_loc + 1) * 128])
                    nc.sync.dma_start(out_d.ap()[b * 128:(b + 1) * 128, :],
                                      acc[:])
                continue

            # einsum + gating + residual per block
            for b_loc in range(nb):
                b = s * SB_BLOCKS + b_loc
                hiT_b = hiT_t[:, b_loc * 128:(b_loc + 1) * 128]
                xT_b = xT_t[:, b * 128:(b + 1) * 128]
                pa = psmm.tile([128, 512], F32, tag="pmm")
                pb = psmm.tile([128, 512], F32, tag="pmm")
                nc.tensor.matmul(pa[:], hiT_b, w_t[:, 0:512],
                                 start=True, stop=False)
                nc.tensor.matmul(pb[:], hiT_b, w_t[:, 512:1024],
                                 start=True, stop=False)
                nc.tensor.matmul(pa[:], xT_b, w_t[:, 1024:1536],
                                 start=False, stop=True)
                nc.tensor.matmul(pb[:], xT_b, w_t[:, 1536:2048],
                                 start=False, stop=True)
                acc = accp.tile([128, F], F32, tag="acc")
                nc.sync.dma_start(acc[:], x_res_d.ap()[b * 128:(b + 1) * 128, :])
                for k in range(K):
                    src = pa if k < 4 else pb
                    kk = k % 4
                    nc.vector.scalar_tensor_tensor(
                        acc[:], src[:, kk * 128:(kk + 1) * 128],
                        eg_t[:, b * K + k:b * K + k + 1], acc[:],
                        mybir.AluOpType.mult, mybir.AluOpType.add)
                nc.sync.dma_start(out_d.ap()[b * 128:(b + 1) * 128, :], acc[:])

    nc.compile()
    return nc


def _bench(nc, in_maps, n_cores, k_lo=2, k_hi=16, reps=3):
    """Amortized per-execution wall time of the compiled NEFF on the axon
    cores (inputs staged on device, pipelined async dispatches). Upper
    bound: includes axon per-dispatch overhead (~2.5-3 ms amortized).
    Returns (per_exec_ns, results_list)."""
    import time

    import jax
    from jax.sharding import Mesh, PartitionSpec
    from jax.experimental.shard_map import shard_map

    from concourse import bass2jax, mybir as _mb
    from concourse.bass2jax import _bass_exec_p, partition_id_tensor

    bass2jax.install_neuronx_cc_hook()

    partition_name = (nc.partition_id_tensor.name
                      if nc.partition_id_tensor else None)
    in_names, out_names, out_avals, zero_outs = [], [], [], []
    for alloc in nc.m.functions[0].allocations:
        if not isinstance(alloc, _mb.MemoryLocationSet):
            continue
        name = alloc.memorylocations[0].name
        if alloc.kind == "ExternalInput":
            if name != partition_name:
                in_names.append(name)
        elif alloc.kind == "ExternalOutput":
            shape = tuple(alloc.tensor_shape)
            dtype = _mb.dt.np(alloc.dtype)
            out_names.append(name)
            out_avals.append(jax.core.ShapedArray(shape, dtype))
            zero_outs.append(np.zeros(shape, dtype))
    n_params = len(in_names)
    all_in_names = in_names + out_names
    if partition_name is not None:
        all_in_names = all_in_names + [partition_name]
    def _body(*args):
        operands = list(args)
        if partition_name is not None:
            operands.append(partition_id_tensor())
        return tuple(_bass_exec_p.bind(
            *operands, out_avals=tuple(out_avals),
            in_names=tuple(all_in_names), out_names=tuple(out_names),
            lowering_input_output_aliases=(), sim_require_finite=True,
            sim_require_nnan=True, nc=nc))

    devices = jax.devices()[:n_cores]
    mesh = Mesh(np.asarray(devices), ("core",))
    nin = n_params + len(out_names)
    sh = jax.sharding.NamedSharding(mesh, PartitionSpec("core"))
    concat_in = [jax.device_put(
        np.concatenate([np.asarray(in_maps[c][k]) for c in range(n_cores)], 0),
        sh) for k in in_names]
    concat_zeros = [jax.device_put(
        np.zeros((n_cores * z.shape[0], *z.shape[1:]), z.dtype), sh)
        for z in zero_outs]
    fn = jax.jit(shard_map(_body, mesh=mesh,
                           in_specs=(PartitionSpec("core"),) * nin,
                           out_specs=(PartitionSpec("core"),) * len(out_names),
                           check_rep=False), keep_unused=True)
    out = fn(*concat_in, *concat_zeros)   # warmup (compile+load)
    jax.block_until_ready(out)
    iters = k_hi
    best = float("inf")
    for _ in range(reps):
        t0 = time.perf_counter()
        for _ in range(iters):
            out = fn(*concat_in, *concat_zeros)
        jax.block_until_ready(out)
        best = min(best, (time.perf_counter() - t0) / iters)
    results = [{name: np.asarray(out[i]).reshape(n_cores, *out_avals[i].shape)[c]
                for i, name in enumerate(out_names)} for c in range(n_cores)]
    return best * 1e9, results


def _run(x, adj, e, weights, n_cores=8, sim=False, trace=False):
    meta, in_maps = _prep(x, adj, e, weights, n_cores)
    nc = _build(meta)
    N, F, NPC, NPCP = meta["N"], meta["F"], meta["NPC"], meta["NPCP"]

    if sim:
        from concourse.bass_interp import CoreSim
        outs = []
        for c in range(n_cores):
            simr = CoreSim(nc)
            for k, v in in_maps[c].items():
                simr.tensor(k)[:] = v
            simr.simulate(check_with_hw=False)
            outs.append(np.array(simr.tensor("out")))
        res = None
    elif trace:
        per_iter_ns, results = _bench(nc, in_maps, n_cores)
        outs = [r["out"] for r in results]
        res = per_iter_ns
    else:
        from concourse.bass_utils import run_bass_kernel_spmd
        res = run_bass_kernel_spmd(nc, in_maps, core_ids=list(range(n_cores)),
                                   trace=trace)
        outs = [r["out"] for r in res.results]

    out = np.concatenate([o[:NPC] for o in outs], axis=0)
    assert out.shape == (N, F)
    return out.astype(np.float32), res


def kernel(x, adj, e, weights):
    x = np.asarray(x, dtype=np.float32)
    adj = np.asarray(adj)
    e = np.asarray(e, dtype=np.float32)
    weights = np.asarray(weights, dtype=np.float32)
    out, _ = _run(x, adj, e, weights, n_cores=8, sim=False)
    return out



# revision 14
# speedup vs baseline: 1.2792x; 1.2792x over previous
"""CaNetConv (GCN conv + gated multi-head linear) Trainium2 kernel.

Strategy (pull-mode graph SpMM, destinations sharded across 8 cores):
  hi[c,:] = sum_{e: col[e]=c} val[e] * x[row[e],:]      (GCN aggregation)
  out     = x + sum_k e[:,k] * (concat(hi,x) @ W[k])    (gated einsum)

Per core:
  - edges sorted by destination; 128-edge groups; x rows fetched with
    gpsimd dma_gather (512B tokens -> full DMA line rate)
  - DVE builds S[e,c] = (iota==colrel[e]) * val[e] in one tensor_scalar
  - PE computes hiT[f,c] += msg[e,f]^T @ S[e,c] (segment sum as matmul,
    accumulated in PSUM per 128-dest block)
  - einsum as 4 matmuls per 128-node block vs preflattened W, gating sum
    via scalar_tensor_tensor with per-partition e scalars, + residual.

One NEFF shared by all 8 cores (SPMD): the static structure (gather run
lengths, groups per block) is padded to the max across cores.
"""

import sys

import numpy as np

for _p in ("/opt/trn_rl_repo", "/root/.axon_site/_ro/trn_rl_repo"):
    if _p not in sys.path:
        sys.path.append(_p)

import concourse.bass as bass  # noqa: E402
import concourse.tile as tile  # noqa: E402
from concourse import bacc, mybir  # noqa: E402

F32 = mybir.dt.float32
BF16 = mybir.dt.bfloat16
I16 = mybir.dt.int16

try:
    from ml_dtypes import bfloat16 as BF16NP
except ImportError:  # pragma: no cover
    import jax.numpy as _jnp
    BF16NP = _jnp.bfloat16

SUBRUN = 1024          # gather tokens per dma_gather; the SWDGE descriptor
                       # carveout (16KB/partition / 16B per desc) caps one
                       # gather at ~1024 descriptors — 1280 kills the device
SB_BLOCKS = 4          # dest blocks (128 dests) per gather superblock
PAD_COLREL = 200.0     # colrel sentinel that never matches iota 0..127


def _wrap16(a):
    """dma_gather index layout: [128, n/16], idx t at [t%16 (+16g), t//16]."""
    n = a.shape[0]
    assert n % 16 == 0
    w = a.reshape(n // 16, 16).T.astype(np.int16)  # [16, n/16]
    return np.tile(w, (8, 1))                      # replicated per Q7 core


def _prep(x, adj, e, weights, n_cores):
    """Host-side graph preprocessing. Returns (meta, in_maps)."""
    N, F = x.shape
    K = e.shape[1]
    E = adj.shape[1]
    row = np.asarray(adj[0], dtype=np.int64)
    col = np.asarray(adj[1], dtype=np.int64)

    NPC = N // n_cores                      # dests per core
    NB = (NPC + 127) // 128                 # 128-dest blocks per core
    NPCP = NB * 128
    NSB = (NB + SB_BLOCKS - 1) // SB_BLOCKS
    sb_nblocks = [min(SB_BLOCKS, NB - s * SB_BLOCKS) for s in range(NSB)]
    import os as _os
    n_half = 2 if (N > 32767 or _os.environ.get("KERNEL_FORCE_HALVES")) else 1
    HALF = (N + 1) // 2 if n_half == 2 else N

    # GCN normalization (destination degree), f32 like the reference
    deg = np.bincount(col, minlength=N).astype(np.float32)
    with np.errstate(divide="ignore"):
        r = 1.0 / np.sqrt(deg)
    r[~np.isfinite(r)] = 0.0
    val_e = (r[col] * r[row]).astype(np.float32)

    # per-core edge lists sorted by local dest
    cores = []
    counts = np.zeros((n_cores, NB, n_half), dtype=np.int64)
    for c in range(n_cores):
        m = (col >= c * NPC) & (col < (c + 1) * NPC)
        rc = row[m]
        cc = col[m] - c * NPC
        vc = val_e[m]
        o = np.argsort(cc, kind="stable")
        rc, cc, vc = rc[o], cc[o], vc[o]
        blk = cc >> 7
        half = (rc >= HALF).astype(np.int64)
        # bucket edges per (block, half), preserving order
        key = blk * n_half + half
        ob = np.argsort(key, kind="stable")
        rc, cc, vc, blk, half = rc[ob], cc[ob], vc[ob], blk[ob], half[ob]
        np.add.at(counts[c], (blk, half), 1)
        cores.append((rc, cc, vc, blk, half))

    # static structure: groups per (block, half) = max over cores
    G = np.maximum.reduce([np.ceil(counts[c] / 128.0).astype(np.int64)
                           for c in range(n_cores)])
    for b in range(NB):
        if G[b].sum() == 0:
            G[b, 0] = 1  # guarantee every block's psum region is written

    # Token stream AND matmul emission are block-major (for b: for h:
    # groups): each (block, half) span is one gather run (single source
    # half), each block's PSUM accumulation group is contiguous, and msg
    # tiles are consumed in stream order (bounded pool pressure).
    sched = []          # per sb: [(token_gidx, b_loc, start, stop), ...]
    sb_runs = []        # per sb: [(h, ntokens), ...] gather runs in order
    for s in range(NSB):
        blocks = list(range(s * SB_BLOCKS, s * SB_BLOCKS + sb_nblocks[s]))
        groups = []
        runs = []
        for bi, b in enumerate(blocks):
            # alternate the half order per block so adjacent blocks' runs
            # share a source half and merge into one gather run below
            order = ((0, 1) if (bi % 2 == 0 or n_half == 1) else (1, 0))[:n_half]
            metas = []
            for h in order:
                if G[b, h] > 0:
                    if runs and runs[-1][0] == h:
                        runs[-1][1] += int(G[b, h]) * 128
                    else:
                        runs.append([h, int(G[b, h]) * 128])
                for _ in range(G[b, h]):
                    metas.append([len(groups) + len(metas),
                                  b - s * SB_BLOCKS, False, False])
            metas[0][2] = True
            metas[-1][3] = True
            groups.extend(metas)
        sched.append(groups)
        sb_runs.append(runs)

    G_total = sum(len(g) for g in sched)

    # per-core token arrays in schedule order
    in_maps = []
    xpad = np.zeros((N + 128, F), dtype=np.float32)
    xpad[:N] = x
    epad = np.zeros((N + 128, K), dtype=np.float32)
    epad[:N] = e
    WF = np.ascontiguousarray(
        weights.astype(np.float32).transpose(1, 0, 2).reshape(2 * F, K * F))
    W_dram = np.concatenate([WF[:F], WF[F:]], axis=1)  # [128, 2*K*F]

    for c in range(n_cores):
        rc, cc, vc, blk, half = cores[c]
        idx_parts, colrel_parts, val_parts = [], [], []
        for s in range(NSB):
            blocks = range(s * SB_BLOCKS, s * SB_BLOCKS + sb_nblocks[s])
            for bi, b in enumerate(blocks):
                for h in ((0, 1) if (bi % 2 == 0 or n_half == 1) else (1, 0))[:n_half]:
                    m = (blk == b) & (half == h)
                    ridx = rc[m] - h * HALF
                    crel = (cc[m] - b * 128).astype(np.float32)
                    v = vc[m]
                    # sort by source row: monotonic gather addresses keep
                    # the DMA access pattern DRAM-friendly (random 512B
                    # reads over the full x cost ~4x on HW)
                    o_src = np.argsort(ridx, kind="stable")
                    ridx, crel, v = ridx[o_src], crel[o_src], v[o_src]
                    n = m.sum()
                    npad = G[b, h] * 128 - n
                    assert npad >= 0
                    idx_parts.append(np.concatenate(
                        [ridx, np.zeros(npad, np.int64)]).astype(np.int16))
                    colrel_parts.append(np.concatenate(
                        [crel, np.full(npad, PAD_COLREL, np.float32)]))
                    val_parts.append(np.concatenate(
                        [v, np.zeros(npad, np.float32)]))
        idx_cat = np.concatenate(idx_parts)
        colrel_cat = np.concatenate(colrel_parts)
        val_cat = np.concatenate(val_parts)
        if _os.environ.get("KERNEL_CLAMP_IDX"):
            idx_cat = (idx_cat % int(_os.environ["KERNEL_CLAMP_IDX"])).astype(np.int16)
        assert idx_cat.shape[0] == G_total * 128

        # wrapped idx stream, sliced per (sb, block, half) gather sub-run
        idx_w = []
        off = 0
        for s in range(NSB):
            for _h, ntok in sb_runs[s]:
                rem = ntok
                while rem > 0:
                    take = min(SUBRUN, rem)
                    idx_w.append(_wrap16(idx_cat[off:off + take]))
                    off += take
                    rem -= take
        idx_dram = (np.concatenate(idx_w, axis=1) if idx_w
                    else np.zeros((128, 16), np.int16))

        colrel_dram = np.ascontiguousarray(
            colrel_cat.reshape(G_total, 128).T)
        val_dram = np.ascontiguousarray(val_cat.reshape(G_total, 128).T)

        xT = np.ascontiguousarray(xpad[c * NPC:c * NPC + NPCP].T)
        x_res = np.ascontiguousarray(xpad[c * NPC:c * NPC + NPCP])
        e_gate = np.ascontiguousarray(
            epad[c * NPC:c * NPC + NPCP].reshape(NB, 128, K)
            .transpose(1, 0, 2).reshape(128, NB * K))

        in_maps.append({
            "x_full": np.ascontiguousarray(xpad).astype(BF16NP),
            "xT": xT.astype(BF16NP),
            "x_res": x_res,
            "e_gate": e_gate,
            "W": np.ascontiguousarray(W_dram).astype(BF16NP),
            "iota": np.tile(np.arange(128, dtype=np.float32),
                            (128, 1)).astype(BF16NP),
            "idx": np.ascontiguousarray(idx_dram),
            "colrel": colrel_dram,
            "val": val_dram,
            "chain": np.zeros((1, 128), np.float32),
        })

    meta = dict(N=N, F=F, K=K, E=E, NPC=NPC, NB=NB, NPCP=NPCP, NSB=NSB,
                sb_nblocks=sb_nblocks, n_half=n_half, HALF=HALF,
                sched=sched, sb_runs=sb_runs, G_total=G_total,
                idx_cols=in_maps[0]["idx"].shape[1],
                n_cores=n_cores)
    return meta, in_maps


def _build(meta):
    """Trace the Bass/Tile kernel for the static structure in meta."""
    from contextlib import ExitStack

    N, F, K = meta["N"], meta["F"], meta["K"]
    NB, NPCP, NSB = meta["NB"], meta["NPCP"], meta["NSB"]
    sb_nblocks, n_half, HALF = meta["sb_nblocks"], meta["n_half"], meta["HALF"]
    sched, sb_runs = meta["sched"], meta["sb_runs"]

    nc = bacc.Bacc("TRN2", target_bir_lowering=False, debug=False,
                   num_devices=meta["n_cores"], num_swdge_queues=4)

    x_full = nc.dram_tensor("x_full", [N + 128, F], BF16, kind="ExternalInput")
    xT_d = nc.dram_tensor("xT", [128, NPCP], BF16, kind="ExternalInput")
    x_res_d = nc.dram_tensor("x_res", [NPCP, F], F32, kind="ExternalInput")
    e_gate_d = nc.dram_tensor("e_gate", [128, NB * K], F32,
                              kind="ExternalInput")
    KF = K * F
    W_d = nc.dram_tensor("W", [128, 2 * KF], BF16, kind="ExternalInput")
    iota_d = nc.dram_tensor("iota", [128, 128], BF16, kind="ExternalInput")
    idx_d = nc.dram_tensor("idx", [128, meta["idx_cols"]], I16,
                           kind="ExternalInput")
    colrel_d = nc.dram_tensor("colrel", [128, meta["G_total"]], F32,
                              kind="ExternalInput")
    val_d = nc.dram_tensor("val", [128, meta["G_total"]], F32,
                           kind="ExternalInput")
    out_d = nc.dram_tensor("out", [NPCP, F], F32, kind="ExternalOutput")
    # tiny chain tensors so a benchmark can sequence K executions of this
    # NEFF inside one jit call (data dependency defeats CSE/DCE)
    chain_i = nc.dram_tensor("chain", [1, 128], F32, kind="ExternalInput")
    chain_o = nc.dram_tensor("chain_out", [1, 128], F32,
                             kind="ExternalOutput")

    W_COLS = 2 * KF
    assert KF == 1024 and F == 128, "einsum slicing hardcoded for K=8, F=128"

    with tile.TileContext(nc) as tc, ExitStack() as ctx:
        const = ctx.enter_context(tc.tile_pool(name="const", bufs=1))
        msgp = ctx.enter_context(tc.tile_pool(name="msg", bufs=10))
        sp = ctx.enter_context(tc.tile_pool(name="sp", bufs=16))
        hiTp = ctx.enter_context(tc.tile_pool(name="hiT", bufs=NSB))
        accp = ctx.enter_context(tc.tile_pool(name="acc", bufs=6))
        psag = ctx.enter_context(tc.tile_pool(name="psag", bufs=3,
                                              space="PSUM"))
        psmm = ctx.enter_context(tc.tile_pool(name="psmm", bufs=4,
                                              space="PSUM"))

        # idx cols consumed by each superblock's gathers (for chunked loads)
        sb_idx_cols = []
        for s in range(NSB):
            cols = 0
            for _h, ntok in sb_runs[s]:
                rem = ntok
                while rem > 0:
                    take = min(SUBRUN, rem)
                    cols += take // 16
                    rem -= take
            sb_idx_cols.append(cols)

        # persistent tiles; idx is loaded in per-superblock chunks (first
        # chunk first) so early gathers don't wait on the full idx stream
        # or the other constant loads.
        idx_t = const.tile([128, meta["idx_cols"]], I16, tag="idx")
        iota_t = const.tile([128, 128], BF16, tag="iota")
        cr_t = const.tile([128, meta["G_total"]], F32, tag="cr")
        val_t = const.tile([128, meta["G_total"]], F32, tag="val")
        w_t = const.tile([128, W_COLS], BF16, tag="w")
        xT_t = const.tile([128, NPCP], BF16, tag="xT")
        eg_t = const.tile([128, NB * K], F32, tag="eg")
        off = 0
        for s in range(NSB):
            if sb_idx_cols[s]:
                nc.sync.dma_start(idx_t[:, off:off + sb_idx_cols[s]],
                                  idx_d.ap()[:, off:off + sb_idx_cols[s]])
                off += sb_idx_cols[s]
            if s == 0:
                nc.sync.dma_start(iota_t[:], iota_d.ap()[:, :])
                nc.sync.dma_start(cr_t[:], colrel_d.ap()[:, :])
                nc.sync.dma_start(val_t[:], val_d.ap()[:, :])
                nc.sync.dma_start(w_t[:], W_d.ap()[:, :])
                nc.sync.dma_start(xT_t[:], xT_d.ap()[:, :])
                nc.sync.dma_start(eg_t[:], e_gate_d.ap()[:, :])

        x_half = [x_full.ap()[h * HALF:N + 128, :] for h in range(n_half)]

        chp = ctx.enter_context(tc.tile_pool(name="chp", bufs=1))
        ch_t = chp.tile([1, 128], F32, tag="ch")
        nc.sync.dma_start(ch_t[:], chain_i.ap()[:, :])
        nc.sync.dma_start(chain_o.ap()[:, :], ch_t[:])

        import os as _os
        _gather_only = _os.environ.get("KERNEL_GATHER_ONLY")
        _plain_dma = _os.environ.get("KERNEL_PLAIN_DMA")

        g_base = 0
        idx_off = 0
        n_gathers = 0
        for s in range(NSB):
            nb = sb_nblocks[s]
            groups = sched[s]
            ps_hi = psag.tile([128, nb * 128], F32, tag="psag")

            # gather msg tiles: one run per (block, half), in SUBRUN chunks
            tok_map = []            # token-order group idx -> (tile, slot)
            for h, ntok in sb_runs[s]:
                rem = ntok
                while rem > 0:
                    take = min(SUBRUN, rem)
                    mt = msgp.tile([128, take // 128, 128], BF16, tag="msg")
                    if _plain_dma:
                        src0 = (idx_off * 16) % (N - take)
                        nc.sync.dma_start(
                            mt[:], x_full.ap()[src0:src0 + take, :]
                            .rearrange("(p t) f -> p t f", p=128))
                    else:
                        nc.gpsimd.dma_gather(
                            mt[:], x_half[h],
                            idx_t[:, idx_off:idx_off + take // 16],
                            take, take, F, queue_num=n_gathers % 4)
                    n_gathers += 1
                    for j in range(take // 128):
                        tok_map.append((mt, j))
                    idx_off += take // 16
                    rem -= take

            if _gather_only:
                for b_loc in range(nb):
                    b = s * SB_BLOCKS + b_loc
                    acc = accp.tile([128, F], F32, tag="acc")
                    nc.vector.tensor_copy(acc[:], iota_t[:])
                    nc.sync.dma_start(out_d.ap()[b * 128:(b + 1) * 128, :],
                                      acc[:])
                g_base += len(groups)
                continue

            # S build + aggregation matmuls, block-major emission order
            for tg, b_loc, start, stop in groups:
                g = g_base + tg
                s_t = sp.tile([128, 128], BF16, tag="s")
                nc.vector.tensor_scalar(
                    s_t[:], iota_t[:], cr_t[:, g:g + 1], val_t[:, g:g + 1],
                    mybir.AluOpType.is_equal, mybir.AluOpType.mult)
                mt, j = tok_map[tg]
                nc.tensor.matmul(
                    ps_hi[:, b_loc * 128:(b_loc + 1) * 128],
                    mt[:, j:j + 1, :], s_t[:], start=start, stop=stop)
            g_base += len(groups)

            hiT_t = hiTp.tile([128, nb * 128], BF16, tag="hiT")
            nc.vector.tensor_copy(hiT_t[:], ps_hi[:])

            import os as _os
            if _os.environ.get("KERNEL_SKIP_EINSUM"):
                for b_loc in range(nb):
                    b = s * SB_BLOCKS + b_loc
                    acc = accp.tile([128, F], F32, tag="acc")
                    nc.vector.tensor_copy(acc[:],
                                          hiT_t[:, b_loc * 128:(b_loc + 1) * 128])
                    nc.sync.dma_start(out_d.ap()[b * 128:(b + 1) * 128, :],
                                      acc[:])
                continue

            # einsum + gating + residual per block
            for b_loc in range(nb):
                b = s * SB_BLOCKS + b_loc
                hiT_b = hiT_t[:, b_loc * 128:(b_loc + 1) * 128]
                xT_b = xT_t[:, b * 128:(b + 1) * 128]
                pa = psmm.tile([128, 512], F32, tag="pmm")
                pb = psmm.tile([128, 512], F32, tag="pmm")
                nc.tensor.matmul(pa[:], hiT_b, w_t[:, 0:512],
                                 start=True, stop=False)
                nc.tensor.matmul(pb[:], hiT_b, w_t[:, 512:1024],
                                 start=True, stop=False)
                nc.tensor.matmul(pa[:], xT_b, w_t[:, 1024:1536],
                                 start=False, stop=True)
                nc.tensor.matmul(pb[:], xT_b, w_t[:, 1536:2048],
                                 start=False, stop=True)
                acc = accp.tile([128, F], F32, tag="acc")
                nc.sync.dma_start(acc[:], x_res_d.ap()[b * 128:(b + 1) * 128, :])
                for k in range(K):
                    src = pa if k < 4 else pb
                    kk = k % 4
                    nc.vector.scalar_tensor_tensor(
                        acc[:], src[:, kk * 128:(kk + 1) * 128],
                        eg_t[:, b * K + k:b * K + k + 1], acc[:],
                        mybir.AluOpType.mult, mybir.AluOpType.add)
                nc.sync.dma_start(out_d.ap()[b * 128:(b + 1) * 128, :], acc[:])

    nc.compile()
    return nc


def _bench(nc, in_maps, n_cores, k_lo=2, k_hi=16, reps=3):
    """Amortized per-execution wall time of the compiled NEFF on the axon
    cores (inputs staged on device, pipelined async dispatches). Upper
    bound: includes axon per-dispatch overhead (~2.5-3 ms amortized).
    Returns (per_exec_ns, results_list)."""
    import time

    import jax
    from jax.sharding import Mesh, PartitionSpec
    from jax.experimental.shard_map import shard_map

    from concourse import bass2jax, mybir as _mb
    from concourse.bass2jax import _bass_exec_p, partition_id_tensor

    bass2jax.install_neuronx_cc_hook()

    partition_name = (nc.partition_id_tensor.name
                      if nc.partition_id_tensor else None)
    in_names, out_names, out_avals, zero_outs = [], [], [], []
    for alloc in nc.m.functions[0].allocations:
        if not isinstance(alloc, _mb.MemoryLocationSet):
            continue
        name = alloc.memorylocations[0].name
        if alloc.kind == "ExternalInput":
            if name != partition_name:
                in_names.append(name)
        elif alloc.kind == "ExternalOutput":
            shape = tuple(alloc.tensor_shape)
            dtype = _mb.dt.np(alloc.dtype)
            out_names.append(name)
            out_avals.append(jax.core.ShapedArray(shape, dtype))
            zero_outs.append(np.zeros(shape, dtype))
    n_params = len(in_names)
    all_in_names = in_names + out_names
    if partition_name is not None:
        all_in_names = all_in_names + [partition_name]
    def _body(*args):
        operands = list(args)
        if partition_name is not None:
            operands.append(partition_id_tensor())
        return tuple(_bass_exec_p.bind(
            *operands, out_avals=tuple(out_avals),
            in_names=tuple(all_in_names), out_names=tuple(out_names),
            lowering_input_output_aliases=(), sim_require_finite=True,
            sim_require_nnan=True, nc=nc))

    devices = jax.devices()[:n_cores]
    mesh = Mesh(np.asarray(devices), ("core",))
    nin = n_params + len(out_names)
    sh = jax.sharding.NamedSharding(mesh, PartitionSpec("core"))
    concat_in = [jax.device_put(
        np.concatenate([np.asarray(in_maps[c][k]) for c in range(n_cores)], 0),
        sh) for k in in_names]
    concat_zeros = [jax.device_put(
        np.zeros((n_cores * z.shape[0], *z.shape[1:]), z.dtype), sh)
        for z in zero_outs]
    fn = jax.jit(shard_map(_body, mesh=mesh,
                           in_specs=(PartitionSpec("core"),) * nin,
                           out_specs=(PartitionSpec("core"),) * len(out_names),
                           check_rep=False), keep_unused=True)
    out = fn(*concat_in, *concat_zeros)   # warmup (compile+load)
    jax.block_until_ready(out)
    iters = k_hi
    best = float("inf")
    for _ in range(reps):
        t0 = time.perf_counter()
        for _ in range(iters):
            out = fn(*concat_in, *concat_zeros)
        jax.block_until_ready(out)
        best = min(best, (time.perf_counter() - t0) / iters)
    results = [{name: np.asarray(out[i]).reshape(n_cores, *out_avals[i].shape)[c]
                for i, name in enumerate(out_names)} for c in range(n_cores)]
    return best * 1e9, results


def _run(x, adj, e, weights, n_cores=8, sim=False, trace=False):
    meta, in_maps = _prep(x, adj, e, weights, n_cores)
    nc = _build(meta)
    N, F, NPC, NPCP = meta["N"], meta["F"], meta["NPC"], meta["NPCP"]

    if sim:
        from concourse.bass_interp import CoreSim
        outs = []
        for c in range(n_cores):
            simr = CoreSim(nc)
            for k, v in in_maps[c].items():
                simr.tensor(k)[:] = v
            simr.simulate(check_with_hw=False)
            outs.append(np.array(simr.tensor("out")))
        res = None
    elif trace:
        per_iter_ns, results = _bench(nc, in_maps, n_cores)
        outs = [r["out"] for r in results]
        res = per_iter_ns
    else:
        from concourse.bass_utils import run_bass_kernel_spmd
        res = run_bass_kernel_spmd(nc, in_maps, core_ids=list(range(n_cores)),
                                   trace=trace)
        outs = [r["out"] for r in res.results]

    out = np.concatenate([o[:NPC] for o in outs], axis=0)
    assert out.shape == (N, F)
    return out.astype(np.float32), res


def kernel(x, adj, e, weights):
    x = np.asarray(x, dtype=np.float32)
    adj = np.asarray(adj)
    e = np.asarray(e, dtype=np.float32)
    weights = np.asarray(weights, dtype=np.float32)
    out, _ = _run(x, adj, e, weights, n_cores=8, sim=False)
    return out



# revision 18
# speedup vs baseline: 1.3804x; 1.0791x over previous
"""CaNetConv (GCN conv + gated multi-head linear) Trainium2 kernel.

Strategy (pull-mode graph SpMM, destinations sharded across 8 cores):
  hi[c,:] = sum_{e: col[e]=c} val[e] * x[row[e],:]      (GCN aggregation)
  out     = x + sum_k e[:,k] * (concat(hi,x) @ W[k])    (gated einsum)

Per core:
  - edges sorted by destination; 128-edge groups; x rows fetched with
    gpsimd dma_gather (512B tokens -> full DMA line rate)
  - DVE builds S[e,c] = (iota==colrel[e]) * val[e] in one tensor_scalar
  - PE computes hiT[f,c] += msg[e,f]^T @ S[e,c] (segment sum as matmul,
    accumulated in PSUM per 128-dest block)
  - einsum as 4 matmuls per 128-node block vs preflattened W, gating sum
    via scalar_tensor_tensor with per-partition e scalars, + residual.

One NEFF shared by all 8 cores (SPMD): the static structure (gather run
lengths, groups per block) is padded to the max across cores.
"""

import sys

import numpy as np

for _p in ("/opt/trn_rl_repo", "/root/.axon_site/_ro/trn_rl_repo"):
    if _p not in sys.path:
        sys.path.append(_p)

import concourse.bass as bass  # noqa: E402
import concourse.tile as tile  # noqa: E402
from concourse import bacc, mybir  # noqa: E402

F32 = mybir.dt.float32
BF16 = mybir.dt.bfloat16
I16 = mybir.dt.int16

try:
    from ml_dtypes import bfloat16 as BF16NP
except ImportError:  # pragma: no cover
    import jax.numpy as _jnp
    BF16NP = _jnp.bfloat16

SUBRUN = 1024          # gather tokens per dma_gather; the SWDGE descriptor
                       # carveout (16KB/partition / 16B per desc) caps one
                       # gather at ~1024 descriptors — 1280 kills the device
SB_BLOCKS = 4          # dest blocks (128 dests) per gather superblock
PAD_COLREL = 200.0     # colrel sentinel that never matches iota 0..127


def _wrap16(a):
    """dma_gather index layout: [128, n/16], idx t at [t%16 (+16g), t//16]."""
    n = a.shape[0]
    assert n % 16 == 0
    w = a.reshape(n // 16, 16).T.astype(np.int16)  # [16, n/16]
    return np.tile(w, (8, 1))                      # replicated per Q7 core


def _prep(x, adj, e, weights, n_cores):
    """Host-side graph preprocessing. Returns (meta, in_maps)."""
    N, F = x.shape
    K = e.shape[1]
    E = adj.shape[1]
    row = np.asarray(adj[0], dtype=np.int64)
    col = np.asarray(adj[1], dtype=np.int64)

    NPC = N // n_cores                      # dests per core
    NB = (NPC + 127) // 128                 # 128-dest blocks per core
    NPCP = NB * 128
    NSB = (NB + SB_BLOCKS - 1) // SB_BLOCKS
    sb_nblocks = [min(SB_BLOCKS, NB - s * SB_BLOCKS) for s in range(NSB)]
    import os as _os
    n_half = 2 if (N > 32767 or _os.environ.get("KERNEL_FORCE_HALVES")) else 1
    HALF = (N + 1) // 2 if n_half == 2 else N

    # GCN normalization (destination degree), f32 like the reference
    deg = np.bincount(col, minlength=N).astype(np.float32)
    with np.errstate(divide="ignore"):
        r = 1.0 / np.sqrt(deg)
    r[~np.isfinite(r)] = 0.0
    val_e = (r[col] * r[row]).astype(np.float32)

    # per-core edge lists sorted by local dest
    cores = []
    counts = np.zeros((n_cores, NB, n_half), dtype=np.int64)
    for c in range(n_cores):
        m = (col >= c * NPC) & (col < (c + 1) * NPC)
        rc = row[m]
        cc = col[m] - c * NPC
        vc = val_e[m]
        o = np.argsort(cc, kind="stable")
        rc, cc, vc = rc[o], cc[o], vc[o]
        blk = cc >> 7
        half = (rc >= HALF).astype(np.int64)
        # bucket edges per (block, half), preserving order
        key = blk * n_half + half
        ob = np.argsort(key, kind="stable")
        rc, cc, vc, blk, half = rc[ob], cc[ob], vc[ob], blk[ob], half[ob]
        np.add.at(counts[c], (blk, half), 1)
        cores.append((rc, cc, vc, blk, half))

    # static structure: groups per (block, half) = max over cores
    G = np.maximum.reduce([np.ceil(counts[c] / 128.0).astype(np.int64)
                           for c in range(n_cores)])
    for b in range(NB):
        if G[b].sum() == 0:
            G[b, 0] = 1  # guarantee every block's psum region is written

    # Token stream AND matmul emission are block-major (for b: for h:
    # groups): each (block, half) span is one gather run (single source
    # half), each block's PSUM accumulation group is contiguous, and msg
    # tiles are consumed in stream order (bounded pool pressure).
    sched = []          # per sb: [(token_gidx, b_loc, start, stop), ...]
    sb_runs = []        # per sb: [(h, ntokens), ...] gather runs in order
    for s in range(NSB):
        blocks = list(range(s * SB_BLOCKS, s * SB_BLOCKS + sb_nblocks[s]))
        groups = []
        runs = []
        for bi, b in enumerate(blocks):
            # alternate the half order per block so adjacent blocks' runs
            # share a source half and merge into one gather run below
            order = ((0, 1) if (bi % 2 == 0 or n_half == 1) else (1, 0))[:n_half]
            metas = []
            for h in order:
                if G[b, h] > 0:
                    if runs and runs[-1][0] == h:
                        runs[-1][1] += int(G[b, h]) * 128
                    else:
                        runs.append([h, int(G[b, h]) * 128])
                for _ in range(G[b, h]):
                    metas.append([len(groups) + len(metas),
                                  b - s * SB_BLOCKS, False, False])
            metas[0][2] = True
            metas[-1][3] = True
            groups.extend(metas)
        sched.append(groups)
        sb_runs.append(runs)

    G_total = sum(len(g) for g in sched)

    # per-core token arrays in schedule order
    in_maps = []
    xpad = np.zeros((N + 128, F), dtype=np.float32)
    xpad[:N] = x
    epad = np.zeros((N + 128, K), dtype=np.float32)
    epad[:N] = e
    WF = np.ascontiguousarray(
        weights.astype(np.float32).transpose(1, 0, 2).reshape(2 * F, K * F))
    W_dram = np.concatenate([WF[:F], WF[F:]], axis=1)  # [128, 2*K*F]

    for c in range(n_cores):
        rc, cc, vc, blk, half = cores[c]
        idx_parts, colrel_parts, val_parts = [], [], []
        for s in range(NSB):
            blocks = range(s * SB_BLOCKS, s * SB_BLOCKS + sb_nblocks[s])
            for bi, b in enumerate(blocks):
                for h in ((0, 1) if (bi % 2 == 0 or n_half == 1) else (1, 0))[:n_half]:
                    m = (blk == b) & (half == h)
                    ridx = rc[m] - h * HALF
                    crel = (cc[m] - b * 128).astype(np.float32)
                    v = vc[m]
                    # sort by source row: monotonic gather addresses keep
                    # the DMA access pattern DRAM-friendly (random 512B
                    # reads over the full x cost ~4x on HW)
                    o_src = np.argsort(ridx, kind="stable")
                    ridx, crel, v = ridx[o_src], crel[o_src], v[o_src]
                    n = m.sum()
                    npad = G[b, h] * 128 - n
                    assert npad >= 0
                    idx_parts.append(np.concatenate(
                        [ridx, np.zeros(npad, np.int64)]).astype(np.int16))
                    colrel_parts.append(np.concatenate(
                        [crel, np.full(npad, PAD_COLREL, np.float32)]))
                    val_parts.append(np.concatenate(
                        [v, np.zeros(npad, np.float32)]))
        idx_cat = np.concatenate(idx_parts)
        colrel_cat = np.concatenate(colrel_parts)
        val_cat = np.concatenate(val_parts)
        if _os.environ.get("KERNEL_CLAMP_IDX"):
            idx_cat = (idx_cat % int(_os.environ["KERNEL_CLAMP_IDX"])).astype(np.int16)
        assert idx_cat.shape[0] == G_total * 128

        # wrapped idx stream, sliced per (sb, block, half) gather sub-run
        idx_w = []
        off = 0
        for s in range(NSB):
            for _h, ntok in sb_runs[s]:
                rem = ntok
                while rem > 0:
                    take = min(SUBRUN, rem)
                    idx_w.append(_wrap16(idx_cat[off:off + take]))
                    off += take
                    rem -= take
        idx_dram = (np.concatenate(idx_w, axis=1) if idx_w
                    else np.zeros((128, 16), np.int16))

        colrel_dram = np.ascontiguousarray(
            colrel_cat.reshape(G_total, 128).T)
        val_dram = np.ascontiguousarray(val_cat.reshape(G_total, 128).T)

        xT = np.ascontiguousarray(xpad[c * NPC:c * NPC + NPCP].T)
        x_res = np.ascontiguousarray(xpad[c * NPC:c * NPC + NPCP])
        e_gate = np.ascontiguousarray(
            epad[c * NPC:c * NPC + NPCP].reshape(NB, 128, K)
            .transpose(1, 0, 2).reshape(128, NB * K))

        in_maps.append({
            "x_full": np.ascontiguousarray(xpad),
            "xT": xT.astype(BF16NP),
            "x_res": x_res,
            "e_gate": e_gate,
            "W": np.ascontiguousarray(W_dram).astype(BF16NP),
            "iota": np.tile(np.arange(128, dtype=np.float32),
                            (128, 1)).astype(BF16NP),
            "idx": np.ascontiguousarray(idx_dram),
            "colrel": colrel_dram,
            "val": val_dram,
            "chain": np.zeros((1, 128), np.float32),
        })

    meta = dict(N=N, F=F, K=K, E=E, NPC=NPC, NB=NB, NPCP=NPCP, NSB=NSB,
                sb_nblocks=sb_nblocks, n_half=n_half, HALF=HALF,
                sched=sched, sb_runs=sb_runs, G_total=G_total,
                idx_cols=in_maps[0]["idx"].shape[1],
                n_cores=n_cores)
    return meta, in_maps


def _build(meta):
    """Trace the Bass/Tile kernel for the static structure in meta."""
    from contextlib import ExitStack

    N, F, K = meta["N"], meta["F"], meta["K"]
    NB, NPCP, NSB = meta["NB"], meta["NPCP"], meta["NSB"]
    sb_nblocks, n_half, HALF = meta["sb_nblocks"], meta["n_half"], meta["HALF"]
    sched, sb_runs = meta["sched"], meta["sb_runs"]

    nc = bacc.Bacc("TRN2", target_bir_lowering=False, debug=False,
                   num_devices=meta["n_cores"], num_swdge_queues=4)

    x_full = nc.dram_tensor("x_full", [N + 128, F], F32, kind="ExternalInput")
    xT_d = nc.dram_tensor("xT", [128, NPCP], BF16, kind="ExternalInput")
    x_res_d = nc.dram_tensor("x_res", [NPCP, F], F32, kind="ExternalInput")
    e_gate_d = nc.dram_tensor("e_gate", [128, NB * K], F32,
                              kind="ExternalInput")
    KF = K * F
    W_d = nc.dram_tensor("W", [128, 2 * KF], BF16, kind="ExternalInput")
    iota_d = nc.dram_tensor("iota", [128, 128], BF16, kind="ExternalInput")
    idx_d = nc.dram_tensor("idx", [128, meta["idx_cols"]], I16,
                           kind="ExternalInput")
    colrel_d = nc.dram_tensor("colrel", [128, meta["G_total"]], F32,
                              kind="ExternalInput")
    val_d = nc.dram_tensor("val", [128, meta["G_total"]], F32,
                           kind="ExternalInput")
    out_d = nc.dram_tensor("out", [NPCP, F], F32, kind="ExternalOutput")
    # tiny chain tensors so a benchmark can sequence K executions of this
    # NEFF inside one jit call (data dependency defeats CSE/DCE)
    chain_i = nc.dram_tensor("chain", [1, 128], F32, kind="ExternalInput")
    chain_o = nc.dram_tensor("chain_out", [1, 128], F32,
                             kind="ExternalOutput")

    W_COLS = 2 * KF
    assert KF == 1024 and F == 128, "einsum slicing hardcoded for K=8, F=128"

    with tile.TileContext(nc) as tc, ExitStack() as ctx:
        const = ctx.enter_context(tc.tile_pool(name="const", bufs=1))
        msgp = ctx.enter_context(tc.tile_pool(name="msg", bufs=10))
        msgbp = ctx.enter_context(tc.tile_pool(name="msgb", bufs=10))
        sp = ctx.enter_context(tc.tile_pool(name="sp", bufs=16))
        hiTp = ctx.enter_context(tc.tile_pool(name="hiT", bufs=NSB))
        accp = ctx.enter_context(tc.tile_pool(name="acc", bufs=6))
        psag = ctx.enter_context(tc.tile_pool(name="psag", bufs=3,
                                              space="PSUM"))
        psmm = ctx.enter_context(tc.tile_pool(name="psmm", bufs=4,
                                              space="PSUM"))

        # idx cols consumed by each superblock's gathers (for chunked loads)
        sb_idx_cols = []
        for s in range(NSB):
            cols = 0
            for _h, ntok in sb_runs[s]:
                rem = ntok
                while rem > 0:
                    take = min(SUBRUN, rem)
                    cols += take // 16
                    rem -= take
            sb_idx_cols.append(cols)

        # persistent tiles; idx is loaded in per-superblock chunks (first
        # chunk first) so early gathers don't wait on the full idx stream
        # or the other constant loads.
        idx_t = const.tile([128, meta["idx_cols"]], I16, tag="idx")
        iota_t = const.tile([128, 128], BF16, tag="iota")
        cr_t = const.tile([128, meta["G_total"]], F32, tag="cr")
        val_t = const.tile([128, meta["G_total"]], F32, tag="val")
        w_t = const.tile([128, W_COLS], BF16, tag="w")
        xT_t = const.tile([128, NPCP], BF16, tag="xT")
        eg_t = const.tile([128, NB * K], F32, tag="eg")
        off = 0
        for s in range(NSB):
            if sb_idx_cols[s]:
                nc.sync.dma_start(idx_t[:, off:off + sb_idx_cols[s]],
                                  idx_d.ap()[:, off:off + sb_idx_cols[s]])
                off += sb_idx_cols[s]
            if s == 0:
                nc.sync.dma_start(iota_t[:], iota_d.ap()[:, :])
                nc.sync.dma_start(cr_t[:], colrel_d.ap()[:, :])
                nc.sync.dma_start(val_t[:], val_d.ap()[:, :])
                nc.sync.dma_start(w_t[:], W_d.ap()[:, :])
                nc.sync.dma_start(xT_t[:], xT_d.ap()[:, :])
                nc.sync.dma_start(eg_t[:], e_gate_d.ap()[:, :])

        x_half = [x_full.ap()[h * HALF:N + 128, :] for h in range(n_half)]

        chp = ctx.enter_context(tc.tile_pool(name="chp", bufs=1))
        ch_t = chp.tile([1, 128], F32, tag="ch")
        nc.sync.dma_start(ch_t[:], chain_i.ap()[:, :])
        nc.sync.dma_start(chain_o.ap()[:, :], ch_t[:])

        import os as _os
        _gather_only = _os.environ.get("KERNEL_GATHER_ONLY")
        _plain_dma = _os.environ.get("KERNEL_PLAIN_DMA")

        g_base = 0
        idx_off = 0
        n_gathers = 0
        for s in range(NSB):
            nb = sb_nblocks[s]
            groups = sched[s]
            ps_hi = psag.tile([128, nb * 128], F32, tag="psag")

            # gather msg tiles: one run per (block, half), in SUBRUN chunks
            tok_map = []            # token-order group idx -> (tile, slot)
            for h, ntok in sb_runs[s]:
                rem = ntok
                while rem > 0:
                    take = min(SUBRUN, rem)
                    mt = msgp.tile([128, take // 128, 128], F32, tag="msg")
                    if _plain_dma:
                        src0 = (idx_off * 16) % (N - take)
                        nc.sync.dma_start(
                            mt[:], x_full.ap()[src0:src0 + take, :]
                            .rearrange("(p t) f -> p t f", p=128))
                    else:
                        nc.gpsimd.dma_gather(
                            mt[:], x_half[h],
                            idx_t[:, idx_off:idx_off + take // 16],
                            take, take, F, queue_num=n_gathers % 4)
                    n_gathers += 1
                    # cast to bf16 on the (otherwise idle) Act engine so the
                    # aggregation matmuls run at 1 cycle/row instead of 4
                    mb = msgbp.tile([128, take // 128, 128], BF16, tag="msgb")
                    nc.scalar.copy(mb[:], mt[:])
                    for j in range(take // 128):
                        tok_map.append((mb, j))
                    idx_off += take // 16
                    rem -= take

            if _gather_only:
                for b_loc in range(nb):
                    b = s * SB_BLOCKS + b_loc
                    acc = accp.tile([128, F], F32, tag="acc")
                    nc.vector.tensor_copy(acc[:], iota_t[:])
                    nc.sync.dma_start(out_d.ap()[b * 128:(b + 1) * 128, :],
                                      acc[:])
                g_base += len(groups)
                continue

            # S build + aggregation matmuls, block-major emission order
            for tg, b_loc, start, stop in groups:
                g = g_base + tg
                s_t = sp.tile([128, 128], BF16, tag="s")
                nc.vector.tensor_scalar(
                    s_t[:], iota_t[:], cr_t[:, g:g + 1], val_t[:, g:g + 1],
                    mybir.AluOpType.is_equal, mybir.AluOpType.mult)
                mt, j = tok_map[tg]
                nc.tensor.matmul(
                    ps_hi[:, b_loc * 128:(b_loc + 1) * 128],
                    mt[:, j:j + 1, :], s_t[:], start=start, stop=stop)
            g_base += len(groups)

            hiT_t = hiTp.tile([128, nb * 128], BF16, tag="hiT")
            nc.vector.tensor_copy(hiT_t[:], ps_hi[:])

            import os as _os
            if _os.environ.get("KERNEL_SKIP_EINSUM"):
                for b_loc in range(nb):
                    b = s * SB_BLOCKS + b_loc
                    acc = accp.tile([128, F], F32, tag="acc")
                    nc.vector.tensor_copy(acc[:],
                                          hiT_t[:, b_loc * 128:(b_loc + 1) * 128])
                    nc.sync.dma_start(out_d.ap()[b * 128:(b + 1) * 128, :],
                                      acc[:])
                continue

            # einsum + gating + residual per block
            for b_loc in range(nb):
                b = s * SB_BLOCKS + b_loc
                hiT_b = hiT_t[:, b_loc * 128:(b_loc + 1) * 128]
                xT_b = xT_t[:, b * 128:(b + 1) * 128]
                pa = psmm.tile([128, 512], F32, tag="pmm")
                pb = psmm.tile([128, 512], F32, tag="pmm")
                nc.tensor.matmul(pa[:], hiT_b, w_t[:, 0:512],
                                 start=True, stop=False)
                nc.tensor.matmul(pb[:], hiT_b, w_t[:, 512:1024],
                                 start=True, stop=False)
                nc.tensor.matmul(pa[:], xT_b, w_t[:, 1024:1536],
                                 start=False, stop=True)
                nc.tensor.matmul(pb[:], xT_b, w_t[:, 1536:2048],
                                 start=False, stop=True)
                acc = accp.tile([128, F], F32, tag="acc")
                nc.sync.dma_start(acc[:], x_res_d.ap()[b * 128:(b + 1) * 128, :])
                for k in range(K):
                    src = pa if k < 4 else pb
                    kk = k % 4
                    nc.vector.scalar_tensor_tensor(
                        acc[:], src[:, kk * 128:(kk + 1) * 128],
                        eg_t[:, b * K + k:b * K + k + 1], acc[:],
                        mybir.AluOpType.mult, mybir.AluOpType.add)
                nc.sync.dma_start(out_d.ap()[b * 128:(b + 1) * 128, :], acc[:])

    nc.compile()
    return nc


def _bench(nc, in_maps, n_cores, k_lo=2, k_hi=16, reps=3):
    """Amortized per-execution wall time of the compiled NEFF on the axon
    cores (inputs staged on device, pipelined async dispatches). Upper
    bound: includes axon per-dispatch overhead (~2.5-3 ms amortized).
    Returns (per_exec_ns, results_list)."""
    import time

    import jax
    from jax.sharding import Mesh, PartitionSpec
    from jax.experimental.shard_map import shard_map

    from concourse import bass2jax, mybir as _mb
    from concourse.bass2jax import _bass_exec_p, partition_id_tensor

    bass2jax.install_neuronx_cc_hook()

    partition_name = (nc.partition_id_tensor.name
                      if nc.partition_id_tensor else None)
    in_names, out_names, out_avals, zero_outs = [], [], [], []
    for alloc in nc.m.functions[0].allocations:
        if not isinstance(alloc, _mb.MemoryLocationSet):
            continue
        name = alloc.memorylocations[0].name
        if alloc.kind == "ExternalInput":
            if name != partition_name:
                in_names.append(name)
        elif alloc.kind == "ExternalOutput":
            shape = tuple(alloc.tensor_shape)
            dtype = _mb.dt.np(alloc.dtype)
            out_names.append(name)
            out_avals.append(jax.core.ShapedArray(shape, dtype))
            zero_outs.append(np.zeros(shape, dtype))
    n_params = len(in_names)
    all_in_names = in_names + out_names
    if partition_name is not None:
        all_in_names = all_in_names + [partition_name]
    def _body(*args):
        operands = list(args)
        if partition_name is not None:
            operands.append(partition_id_tensor())
        return tuple(_bass_exec_p.bind(
            *operands, out_avals=tuple(out_avals),
            in_names=tuple(all_in_names), out_names=tuple(out_names),
            lowering_input_output_aliases=(), sim_require_finite=True,
            sim_require_nnan=True, nc=nc))

    devices = jax.devices()[:n_cores]
    mesh = Mesh(np.asarray(devices), ("core",))
    nin = n_params + len(out_names)
    sh = jax.sharding.NamedSharding(mesh, PartitionSpec("core"))
    concat_in = [jax.device_put(
        np.concatenate([np.asarray(in_maps[c][k]) for c in range(n_cores)], 0),
        sh) for k in in_names]
    concat_zeros = [jax.device_put(
        np.zeros((n_cores * z.shape[0], *z.shape[1:]), z.dtype), sh)
        for z in zero_outs]
    fn = jax.jit(shard_map(_body, mesh=mesh,
                           in_specs=(PartitionSpec("core"),) * nin,
                           out_specs=(PartitionSpec("core"),) * len(out_names),
                           check_rep=False), keep_unused=True)
    out = fn(*concat_in, *concat_zeros)   # warmup (compile+load)
    jax.block_until_ready(out)
    iters = k_hi
    best = float("inf")
    for _ in range(reps):
        t0 = time.perf_counter()
        for _ in range(iters):
            out = fn(*concat_in, *concat_zeros)
        jax.block_until_ready(out)
        best = min(best, (time.perf_counter() - t0) / iters)
    results = [{name: np.asarray(out[i]).reshape(n_cores, *out_avals[i].shape)[c]
                for i, name in enumerate(out_names)} for c in range(n_cores)]
    return best * 1e9, results


def _run(x, adj, e, weights, n_cores=8, sim=False, trace=False):
    meta, in_maps = _prep(x, adj, e, weights, n_cores)
    nc = _build(meta)
    N, F, NPC, NPCP = meta["N"], meta["F"], meta["NPC"], meta["NPCP"]

    if sim:
        from concourse.bass_interp import CoreSim
        outs = []
        for c in range(n_cores):
            simr = CoreSim(nc)
            for k, v in in_maps[c].items():
                simr.tensor(k)[:] = v
            simr.simulate(check_with_hw=False)
            outs.append(np.array(simr.tensor("out")))
        res = None
    elif trace:
        per_iter_ns, results = _bench(nc, in_maps, n_cores)
        outs = [r["out"] for r in results]
        res = per_iter_ns
    else:
        from concourse.bass_utils import run_bass_kernel_spmd
        res = run_bass_kernel_spmd(nc, in_maps, core_ids=list(range(n_cores)),
                                   trace=trace)
        outs = [r["out"] for r in res.results]

    out = np.concatenate([o[:NPC] for o in outs], axis=0)
    assert out.shape == (N, F)
    return out.astype(np.float32), res


def kernel(x, adj, e, weights):
    x = np.asarray(x, dtype=np.float32)
    adj = np.asarray(adj)
    e = np.asarray(e, dtype=np.float32)
    weights = np.asarray(weights, dtype=np.float32)
    out, _ = _run(x, adj, e, weights, n_cores=8, sim=False)
    return out

